# revision 13
# baseline (speedup 1.0000x reference)
"""Trainium2 Bass kernel for nn_BoothGroupQuant (v4, phase-batched).

Booth/NAF group quantization: q = rne(x*128); NAF-decompose each q into
signed power-of-two digits; per group of 16 consecutive elements keep only
the 8 largest-exponent digits (ties: earliest element); reconstruct and
scale by 1/128.

Identity: with t = 3q, u = t ^ q, the NAF nonzero-digit mask of q is u
(digit at exponent e <-> bit e+1), negative digits at u & q.  Per-group
top-8 via int16 SWAR band counters (4 bands of 3 exponents), grouped
reduces, one segmented inclusive scan per chunk, and a packed guard-bit
compare.  Design range |q| <= 2730.

v4 structure: all per-group ("tiny") work is batched across chunks into
single [P, 256] passes (one tiny phase instead of four), per-chunk full
passes are emitted phase-by-phase so cross-engine dependencies are
covered by other chunks' vector work.  ACT keeps only q/t/amtx/thx/yt;
sm, K3m-mult and NM2 run as cheap 4x tensor_scalar ops on DVE.
"""
import os
import sys

import numpy as np

for _p in ("/opt/trn_rl_repo", "/root/.axon_site/_ro/trn_rl_repo"):
    if os.path.isdir(_p) and _p not in sys.path:
        sys.path.insert(0, _p)

import concourse.bacc as bacc
import concourse.mybir as mybir
from concourse import bass_utils
from concourse.tile import TileContext

N_CORES = 8
FULL_SHAPE = (4, 1024, 32, 32)
N_TOTAL = 4 * 1024 * 32 * 32          # 4194304
N_CORE = N_TOTAL // N_CORES           # 524288
P = 128                               # SBUF partitions
F_TOTAL = N_CORE // P                 # 4096 free elems per partition
CHUNKS = (256, 1792, 1536, 512)
F_CHUNK = max(CHUNKS)
G_TOTAL = F_TOTAL // 16               # 256 groups per partition
SF = 0.0078125

i16 = mybir.dt.int16
f32 = mybir.dt.float32
Alu = mybir.AluOpType
Act = mybir.ActivationFunctionType
AX = mybir.AxisListType

_CACHE = {}


def _build():
    nc = bacc.Bacc("TRN2")
    x_in = nc.dram_tensor("x", [P, F_TOTAL], f32, kind="ExternalInput")
    y_out = nc.dram_tensor("y", [P, F_TOTAL], f32, kind="ExternalOutput")

    offs, goffs, o = [], [], 0
    for fc in CHUNKS:
        offs.append(o)
        goffs.append(o // 16)
        o += fc

    with TileContext(nc) as tc:
        with tc.tile_pool(name="const", bufs=1) as cpool:
            seg = cpool.tile([P, F_CHUNK], i16)
            nc.gpsimd.memset(seg, 1)
            nc.gpsimd.memset(
                seg.rearrange("p (g s) -> p g s", s=16)[:, :, 0:1], 0)

            with tc.tile_pool(name="work", bufs=1) as pool:
                _body(nc, pool, seg, x_in, y_out, offs, goffs)

    nc.compile()
    return nc


def _body(nc, pool, seg, x_in, y_out, offs, goffs):
    V, S = nc.vector, nc.scalar

    def grp(ap):
        return ap.rearrange("p (g s) -> p g s", s=16)

    def per(nm, ci, fc, dt=i16):
        return pool.tile([P, fc], dt, name=f"{nm}{ci}", tag=f"{nm}{ci}",
                         bufs=1)

    def sh(nm, fc, dt=i16):
        return pool.tile([P, fc], dt, name=nm, tag=nm, bufs=2)

    def tcat(nm, dt=i16):
        return pool.tile([P, G_TOTAL], dt, name=nm, tag=nm, bufs=1)

    REc = tcat("REc")
    ROc = tcat("ROc")
    n2c = tcat("n2c")
    n1c = tcat("n1c")

    qs, us, ws, Pms, amtxs, thxs = [], [], [], [], [], []

    # ---- phase A: quantize, NAF mask, band counts, grouped reduces ----
    for ci, fc in enumerate(CHUNKS):
        sl = slice(offs[ci], offs[ci] + fc)
        gsl = slice(goffs[ci], goffs[ci] + fc // 16)
        xt = sh("xt", fc, f32)
        nc.sync.dma_start(out=xt, in_=x_in[:, sl])
        q = per("q", ci, fc)
        S.activation(q, xt, Act.Copy, scale=128.0)
        t = sh("t", fc)
        S.activation(t, q, Act.Copy, scale=3.0)
        u = per("u", ci, fc)
        V.tensor_tensor(u, t, q, Alu.bitwise_xor)

        A = sh("A", fc)
        V.tensor_scalar(A, u, 1, 0x249, Alu.logical_shift_right,
                        Alu.bitwise_and)
        B = sh("B", fc)
        V.tensor_scalar(B, u, 2, 0x249, Alu.logical_shift_right,
                        Alu.bitwise_and)
        C = sh("C", fc)
        V.tensor_scalar(C, u, 3, 0x249, Alu.logical_shift_right,
                        Alu.bitwise_and)
        V.tensor_tensor(A, A, B, Alu.add)
        V.tensor_tensor(A, A, C, Alu.add)
        D0 = sh("D0", fc)
        V.tensor_scalar(D0, A, 0x1C7, None, Alu.bitwise_and)
        D1 = sh("D1", fc)
        V.tensor_scalar(D1, A, 3, 0x1C7, Alu.logical_shift_right,
                        Alu.bitwise_and)
        with nc.allow_low_precision(reason="exact small int sums"):
            V.tensor_reduce(REc[:, gsl], grp(D0), AX.X, Alu.add)
            V.tensor_reduce(ROc[:, gsl], grp(D1), AX.X, Alu.add)
        qs.append(q)
        us.append(u)

    # ---- batched tiny 1: band sums, b*, theta  (single [P,256] pass) ----
    B2 = tcat("B2")
    V.tensor_scalar(B2, REc, 6, None, Alu.logical_shift_right)
    B1 = tcat("B1")
    V.tensor_scalar(B1, ROc, 63, None, Alu.bitwise_and)
    B3 = tcat("B3")
    V.tensor_scalar(B3, ROc, 6, None, Alu.logical_shift_right)
    s2 = tcat("s2")
    V.tensor_tensor(s2, B3, B2, Alu.add)
    s1 = tcat("s1")
    V.tensor_tensor(s1, s2, B1, Alu.add)
    g3 = tcat("g3")
    V.tensor_scalar(g3, B3, 8, None, Alu.is_ge)
    g2 = tcat("g2")
    V.tensor_scalar(g2, s2, 8, None, Alu.is_ge)
    g1 = tcat("g1")
    V.tensor_scalar(g1, s1, 8, None, Alu.is_ge)
    bstar = tcat("bstar")
    V.tensor_tensor(bstar, g3, g2, Alu.add)
    V.tensor_tensor(bstar, bstar, g1, Alu.add)
    # theta = 8 - s1 + (g1*B1 + g2*B2 + g3*B3)
    V.tensor_tensor(g3, g3, B3, Alu.mult)
    V.tensor_tensor(g2, g2, B2, Alu.mult)
    V.tensor_tensor(g1, g1, B1, Alu.mult)
    V.tensor_tensor(g3, g3, g2, Alu.add)
    V.tensor_tensor(g3, g3, g1, Alu.add)
    V.tensor_tensor(g3, g3, s1, Alu.subtract)
    theta = tcat("theta")
    V.tensor_scalar(theta, g3, 8, None, Alu.add)

    # amtx per chunk on ACT (overlaps phase-B vector work)
    for ci, fc in enumerate(CHUNKS):
        gsl = slice(goffs[ci], goffs[ci] + fc // 16)
        amtx = per("amtx", ci, fc)
        bs = bstar[:, gsl]
        S.activation(grp(amtx), bs[:, :, None].broadcast_to(
            (P, fc // 16, 16)), Act.Copy, scale=3.0, bias=1.0)
        amtxs.append(amtx)

    # ---- phase B: shift to band, spread flags, segmented scan ----
    sms = []
    for ci, fc in enumerate(CHUNKS):
        w = per("w", ci, fc)
        V.tensor_tensor(w, us[ci], amtxs[ci], Alu.logical_shift_right)
        w7 = pool.tile([P, fc], i16, name=f"w7{ci}", tag=f"w7{ci}", bufs=1)
        V.tensor_scalar(w7, w, 7, None, Alu.bitwise_and)
        sm = pool.tile([P, fc], i16, name=f"smB{ci}", tag=f"smB{ci}", bufs=1)
        S.activation(sm, w7, Act.Copy, scale=float(0x111))
        ws.append(w)
        sms.append(sm)
    for ci, fc in enumerate(CHUNKS):
        gsl = slice(goffs[ci], goffs[ci] + fc // 16)
        s = sh("s", fc)
        V.tensor_scalar(s, sms[ci], 0x421, None, Alu.bitwise_and)
        Pm = per("Pm", ci, fc)
        V.tensor_tensor_scan(Pm, seg[:, 0:fc], s, 0.0, Alu.mult, Alu.add)
        TPv = grp(Pm)[:, :, 15]
        V.tensor_scalar(n2c[:, gsl], TPv, 10, 31, Alu.logical_shift_right,
                        Alu.bitwise_and)
        V.tensor_scalar(n1c[:, gsl], TPv, 5, 31, Alu.logical_shift_right,
                        Alu.bitwise_and)
        Pms.append(Pm)

    # ---- batched tiny 2: packed thresholds (+0x4210 guard bias folded) ----
    th1 = tcat("th1")
    V.tensor_tensor(th1, theta, n2c, Alu.subtract)
    th0 = tcat("th0")
    V.tensor_tensor(th0, th1, n1c, Alu.subtract)
    th1c = tcat("th1c")
    V.tensor_scalar(th1c, th1, 0, 32, Alu.max, Alu.mult)
    th0c = tcat("th0c")
    V.tensor_scalar(th0c, th0, 0, None, Alu.max)
    t2s = tcat("t2s")
    V.tensor_scalar(t2s, theta, 1024, 0x4210, Alu.mult, Alu.add)
    V.tensor_tensor(th0c, th0c, th1c, Alu.add)
    V.tensor_tensor(th0c, th0c, t2s, Alu.add)

    # thx per chunk on ACT (overlaps phase-C vector work)
    for ci, fc in enumerate(CHUNKS):
        gsl = slice(goffs[ci], goffs[ci] + fc // 16)
        thx = per("thx", ci, fc)
        tv = th0c[:, gsl]
        S.activation(grp(thx), tv[:, :, None].broadcast_to(
            (P, fc // 16, 16)), Act.Copy)
        thxs.append(thx)

    # ---- phase C: packed compare, keep mask, reconstruct ----
    # big chunks first so the tail drains on a small chunk
    corder = sorted(range(len(CHUNKS)), key=lambda c: -CHUNKS[c])
    k2s, K3ms = {}, {}
    for ci in corder:
        fc = CHUNKS[ci]
        X = sh("X", fc)
        V.tensor_tensor(X, thxs[ci], Pms[ci], Alu.subtract)
        k2 = pool.tile([P, fc], i16, name=f"k2{ci}", tag=f"k2{ci}", bufs=1)
        V.tensor_scalar(k2, X, 12, 4, Alu.logical_shift_right,
                        Alu.bitwise_and)
        k01 = sh("k01", fc)
        V.tensor_scalar(k01, X, 4, 0x21, Alu.logical_shift_right,
                        Alu.bitwise_and)
        K3m = pool.tile([P, fc], i16, name=f"K3m{ci}", tag=f"K3m{ci}",
                        bufs=1)
        S.activation(K3m, k01, Act.Copy, scale=float(0x11))
        k2s[ci], K3ms[ci] = k2, K3m
    for ci in corder:
        fc = CHUNKS[ci]
        sl = slice(offs[ci], offs[ci] + fc)
        w, q, amtx, k2, K3m = ws[ci], qs[ci], amtxs[ci], k2s[ci], K3ms[ci]
        V.tensor_scalar(K3m, K3m, 4, 3, Alu.logical_shift_right,
                        Alu.bitwise_and)
        V.tensor_tensor(k2, k2, K3m, Alu.bitwise_or)
        V.tensor_scalar(k2, k2, -8, None, Alu.bitwise_or)
        V.tensor_tensor(w, w, k2, Alu.bitwise_and)           # wk
        V.tensor_tensor(w, w, amtx, Alu.logical_shift_left)  # UK
        V.tensor_tensor(q, w, q, Alu.bitwise_and)            # NM
        V.tensor_scalar(q, q, 2, None, Alu.mult)             # NM2
        V.tensor_tensor(w, w, q, Alu.subtract)               # val
        yt = sh("yt", fc, f32)
        S.activation(yt, w, Act.Copy, scale=SF / 2.0)
        nc.sync.dma_start(out=y_out[:, sl], in_=yt)


def _get_nc():
    if "nc" not in _CACHE:
        _CACHE["nc"] = _build()
    return _CACHE["nc"]


def kernel(x: np.ndarray, _trace: bool = False, _trace_kwargs=None):
    assert x.shape == FULL_SHAPE and x.dtype == np.float32, (x.shape, x.dtype)
    nc = _get_nc()
    flat = np.ascontiguousarray(x).reshape(N_CORES, P, F_TOTAL)
    in_maps = [{"x": flat[i]} for i in range(N_CORES)]
    kw = {}
    if _trace:
        kw = {"trace": True, **(_trace_kwargs or {})}
    res = bass_utils.run_bass_kernel_spmd(
        nc, in_maps, core_ids=list(range(N_CORES)), **kw)
    out = np.stack([res.results[i]["y"] for i in range(N_CORES)], axis=0)
    out = out.reshape(FULL_SHAPE).astype(np.float32)
    if _trace:
        return out, res
    return out
